# revision 9
# baseline (speedup 1.0000x reference)
"""Trainium2 Bass kernel for nn_AttentionProjector (8-core SPMD), v3.

Math: out = softmax(q @ (x@Wk.T).T) @ (x@Wv.T + Wv_b)
Rewritten (FLOP reduction):
    scores = (q @ Wk) @ x.T      (Wk_b cancels in softmax)
    out    = (softmax(scores) @ x) @ Wv.T + Wv_b

Structure (8 cores):
  warm-up : tiny AllReduce at t=0 absorbs the ~100us ncfw first-collective
            cold start while DMAs and phase 1 run.
  phase 1 : q'T slice via Wk[:,ds_j] columns -> AllGather q'T (f32r
            end-to-end, HWDGE readback).
  phase 2 : scores[l, n_j] = q'T.T @ xT_j, f32r, N sharded.
  softmax : LOCAL row max -> p = exp(s - m_j) immediately (bf16); tiny
            AllGather of m_j overlaps phase 3; online-softmax rescale by
            exp(m_j - M) applied to u/s before the AllReduce-add.
  phase 3 : uT_j = x_j.T @ p_j.T in bf16. h0 matmuls+copies are emitted
            BEFORE the factor chain so the tensor/vector FIFOs never wait
            on the m AllGather; factor broadcast via a small DRAM bounce
            (no PE involvement).
  AR-u    : ONE AllReduce-add, bf16, u halves + s' in a single payload.
  phase 4 : out[:, do_j] = (ctxT/S).T @ Wv[do_j,:].T + Wv_b, all bf16.

Precision: score path (qTs, wk, xT, AG-q') stays f32r -- bf16 there flips
softmax argmaxes past tolerance. Values path (x, p, u, Wv) is bf16
(host sim rel 5.5e-3; measured on HW 2.9e-3 in v2).
"""

import numpy as np

L = 256          # query rows
D = 4096         # d_in == d_out
N = 8192         # tokens
NCORES = 8
NS = N // NCORES     # 1024 tokens per core
DS = D // NCORES     # 512 dout per core

LT = L // 128        # 2 l-tiles
DT = D // 128        # 32 d-tiles
NT = NS // 128       # 8 local n-tiles
HT = DT // 2         # 16 d-tiles per u half

_MAX_WAITS = 1


def _split_waits(nc, mybir, bass_rust):
    """Walrus in this container allows only one sync-wait per instruction;
    move excess waits onto preceding same-engine no-ops."""
    for bb in nc.main_func.blocks:
        new_list = []
        for ins in bb.instructions:
            si = ins.sync_info
            waits = list(si.on_wait) if si is not None else []
            if len(waits) > _MAX_WAITS:
                for i in range(_MAX_WAITS, len(waits), _MAX_WAITS):
                    nop = mybir.InstNoOp(name=f"{ins.name}-wsplit{i}", ins=[], outs=[])
                    nop.engine = ins.engine
                    nop.sync_info = bass_rust.SyncInfo(
                        on_wait=waits[i:i + _MAX_WAITS], on_update=[])
                    new_list.append(nop)
                ins.sync_info = bass_rust.SyncInfo(
                    on_wait=waits[:_MAX_WAITS], on_update=si.on_update)
            new_list.append(ins)
        bb.instructions[:] = new_list


_NC = None


def _build(split_waits=True):
    global _NC
    if _NC is not None and split_waits:
        return _NC
    import bass_rust
    import concourse.bass as bass
    import concourse.mybir as mybir
    import concourse.tile as tile
    from concourse.masks import make_identity
    from contextlib import ExitStack

    f32 = mybir.dt.float32
    f32r = mybir.dt.float32r
    bf16 = mybir.dt.bfloat16
    AF = mybir.ActivationFunctionType
    AX = mybir.AxisListType
    ALU = mybir.AluOpType
    RG = [list(range(NCORES))]

    f16 = mybir.dt.float16
    nc = bass.Bass()

    # per-core external I/O
    t_qts = nc.dram_tensor("qTs", [D, L], f16, kind="ExternalInput")
    t_wk = nc.dram_tensor("wk", [D, DS], f16, kind="ExternalInput")
    t_xt = nc.dram_tensor("xT", [D, NS], f16, kind="ExternalInput")
    t_xb = nc.dram_tensor("xb", [NS, D], bf16, kind="ExternalInput")
    t_wvt = nc.dram_tensor("wvT", [D, DS], bf16, kind="ExternalInput")
    t_wvb = nc.dram_tensor("wvb", [1, DS], bf16, kind="ExternalInput")
    t_out = nc.dram_tensor("out", [L, DS], f32, kind="ExternalOutput")

    # collective bounce buffers (input Local, output Shared)
    warm_in = nc.dram_tensor("warm_in", [1, 128], f32)
    warm_out = nc.dram_tensor("warm_out", [NCORES, 128], f32, addr_space="Shared")
    ar_q_in = nc.dram_tensor("ar_q_in", [DS, L], f16)
    ar_q_out = nc.dram_tensor("ar_q_out", [D, L], f16, addr_space="Shared")
    ar_m_in = nc.dram_tensor("ar_m_in", [L, 1], f32)
    ar_m_out = nc.dram_tensor("ar_m_out", [L, 1], f32, addr_space="Shared")
    fac_dram = nc.dram_tensor("fac_dram", [1, L], f32)
    # u: 32 d-tile blocks + one s' block, single AllReduce payload
    ar_u_in = nc.dram_tensor("ar_u_in", [(DT + 1) * 128, L], bf16)
    ar_u_out = nc.dram_tensor("ar_u_out", [(DT + 1) * 128, L], bf16,
                              addr_space="Shared")

    qts_re = t_qts.ap().rearrange("(kt p) l -> p kt l", p=128)   # [128, 32, 256]
    wk_re = t_wk.ap().rearrange("(kt p) d -> p kt d", p=128)     # [128, 32, 512]
    xt_re = t_xt.ap().rearrange("(dt p) n -> p dt n", p=128)     # [128, 32, 1024]
    xb_re = t_xb.ap().rearrange("(nt p) d -> p nt d", p=128)     # [128, 8, 4096]
    wvt_re = t_wvt.ap().rearrange("(dt p) o -> p dt o", p=128)   # [128, 32, 512]
    arq_re = ar_q_in.ap().rearrange("(dt p) l -> p dt l", p=128)
    arqo_re = ar_q_out.ap().rearrange("(dt p) l -> p dt l", p=128)
    aru_re = ar_u_in.ap().rearrange("(t p) l -> p t l", p=128)
    aruo_re = ar_u_out.ap().rearrange("(t p) l -> p t l", p=128)

    with ExitStack() as ctx:
        tc = ctx.enter_context(tile.TileContext(nc))
        const = ctx.enter_context(tc.tile_pool(name="const", bufs=1))
        small = ctx.enter_context(tc.tile_pool(name="small", bufs=1))
        persist = ctx.enter_context(tc.tile_pool(name="persist", bufs=1))

        # ---- warm-up: tiny collective absorbs ncfw cold-start ----------------
        nc.gpsimd.collective_compute(
            "AllGather", ALU.bypass, replica_groups=RG,
            ins=[warm_in.ap().opt()], outs=[warm_out.ap().opt()])

        ident_bf = const.tile([128, 128], bf16)
        make_identity(nc, ident_bf[:])
        ident_f = const.tile([128, 128], f32)
        make_identity(nc, ident_f[:])
        ones1 = const.tile([1, 128], f32)
        nc.vector.memset(ones1[:], 1.0)
        bias_sb = const.tile([128, DS], bf16)
        wvb_sb = const.tile([1, DS], bf16)
        nc.scalar.dma_start(wvb_sb[:], t_wvb.ap())
        ones1b = const.tile([1, 128], bf16)
        nc.vector.memset(ones1b[:], 1.0)
        s_blk = const.tile([128, L], bf16)       # s' payload block (zeros + 2 cols)
        nc.vector.memset(s_blk[:], 0.0)

        # PE clock-gate warm-up during the first input DMAs
        with tc.tile_pool(name="warmps", bufs=1, space="PSUM") as warmps:
            wps = warmps.tile([128, 128], f32)
            for i in range(28):
                nc.tensor.matmul(wps[:], ident_bf[:], ident_bf[:],
                                 start=(i == 0), stop=(i == 27))
            # bias broadcast [1,DS] -> [128,DS] via rank-1 matmul
            bps = warmps.tile([128, DS], f32)
            nc.tensor.matmul(bps[:], ones1b[:], wvb_sb[:], start=True, stop=True)
            nc.vector.tensor_copy(bias_sb[:], bps[:])

        # persistent across phases
        pT = persist.tile([128, NT, L], bf16)        # p.T (0.5MB)

        # streamed values path on the scalar queue (pools opened before ph2xt
        # so they land outside the xT stream's space)
        xb_pool = ctx.enter_context(tc.tile_pool(name="ph3x", bufs=2))
        wv_pool = ctx.enter_context(tc.tile_pool(name="ph4w", bufs=2))
        XBCH = 4                     # d-tiles per xb chunk (1MB)
        WVCH = 8                     # d-tiles per wv chunk (1MB)

        def xb_load(c, eng=None):
            xb_c = xb_pool.tile([128, NT, XBCH * 128], bf16, name="xb_c")
            (eng or nc.sync).dma_start(
                xb_c[:], xb_re[:, :, c * XBCH * 128:(c + 1) * XBCH * 128])
            return xb_c

        def wv_load(c, eng=None):
            wv_c = wv_pool.tile([128, WVCH, DS], bf16, name="wv_c")
            (eng or nc.sync).dma_start(wv_c[:], wvt_re[:, c * WVCH:(c + 1) * WVCH, :])
            return wv_c

        # ---------------- phase 1: q'T partial = Wk[:,ds_j].T @ q.T ----------
        with tc.tile_pool(name="ph1q", bufs=2) as ph1q, \
             tc.tile_pool(name="ph1wk", bufs=4) as ph1wk, \
             tc.tile_pool(name="ph1ps", bufs=1, space="PSUM") as ph1ps:
            def qts_load(qc):
                qts_c = ph1q.tile([128, 8, L], f16, name="qts_c")
                nc.sync.dma_start(qts_c[:], qts_re[:, qc * 8:(qc + 1) * 8, :])
                return qts_c
            qts_cs = [qts_load(0)]
            qpT_loc = ph1q.tile([128, 4, L], f16, name="qpT_loc")
            ps4 = [ph1ps.tile([128, L], f32, name=f"ph1ps{i}") for i in range(4)]
            KCH = 4                                  # k-tiles per wk chunk (1MB)
            for kc in range(DT // KCH):
                wk_c = ph1wk.tile([128, KCH, DS], f16, name="wk_c")
                nc.sync.dma_start(wk_c[:], wk_re[:, kc * KCH:(kc + 1) * KCH, :])
                if kc % 2 == 0 and kc // 2 + 1 < 4:
                    qts_cs.append(qts_load(kc // 2 + 1))
                for i in range(KCH):
                    kt = kc * KCH + i
                    for dtl in range(4):
                        nc.tensor.matmul(
                            ps4[dtl][:], wk_c[:, i, dtl * 128:(dtl + 1) * 128],
                            qts_cs[kt // 8][:, kt % 8, :],
                            start=(kt == 0), stop=(kt == DT - 1))
            for dtl in range(4):
                nc.vector.tensor_copy(qpT_loc[:, dtl, :], ps4[dtl][:])
            nc.scalar.dma_start(arq_re, qpT_loc[:])
            nc.gpsimd.collective_compute(
                "AllGather", ALU.bypass, replica_groups=RG,
                ins=[ar_q_in.ap().opt()], outs=[ar_q_out.ap().opt()])

        # early value-path loads (fill the collective cold-start window)
        xb_cs = {0: xb_load(0, nc.scalar), 1: xb_load(1, nc.scalar)}
        wv_cs = [wv_load(0, nc.scalar), wv_load(1, nc.scalar)]
        # q'T readback: f32 -> f32r cast (SWDGE) on gpsimd, 4 chunks
        qpT = persist.tile([128, DT, L], f16, name="qpT")
        for rc in range(4):
            nc.scalar.dma_start(qpT[:, rc * 8:(rc + 1) * 8, :],
                                arqo_re[:, rc * 8:(rc + 1) * 8, :])

        # ---------------- phase 2: scores[l, n_j] ----------------------------
        XCH = 8                      # d-tiles per xT chunk (4MB)
        m_both = small.tile([128, 2], f32, name="m_both")
        s_both = small.tile([128, 2], f32, name="s_both")
        p_sb = [persist.tile([128, NS], bf16, name=f'p_sb{i}') for i in range(LT)]
        with tc.tile_pool(name="ph2xt", bufs=4) as xt_pool, \
             tc.tile_pool(name="ph2sc", bufs=1, space="PSUM") as scps_pool:
            score_ps = [[scps_pool.tile([128, 512], f32, name=f'score{i}_{k}')
                         for k in range(2)] for i in range(LT)]
            for c in range(DT // XCH):
                xt_c = xt_pool.tile([128, XCH, NS], f16, name="xt_c")
                nc.sync.dma_start(xt_c[:], xt_re[:, c * XCH:(c + 1) * XCH, :])
                for i in range(XCH):
                    dt = c * XCH + i
                    for lt in range(LT):
                        for nch in range(2):
                            nc.tensor.matmul(
                                score_ps[lt][nch][:],
                                qpT[:, dt, lt * 128:(lt + 1) * 128],
                                xt_c[:, i, nch * 512:(nch + 1) * 512],
                                start=(dt == 0), stop=(dt == DT - 1))

            # local row max, ship m_j, p = exp(s - m_j) immediately
            negm = small.tile([128, 2], f32, name="negm")
            for lt in range(LT):
                mtmp = small.tile([128, 1], f32, name=f"mtmp{lt}")
                nc.vector.tensor_reduce(mtmp[:], score_ps[lt][0][:], axis=AX.X, op=ALU.max)
                nc.vector.tensor_reduce(m_both[:, lt:lt + 1], score_ps[lt][1][:],
                                        axis=AX.X, op=ALU.max)
                nc.vector.tensor_tensor(m_both[:, lt:lt + 1], m_both[:, lt:lt + 1],
                                        mtmp[:], ALU.max)
            nc.vector.tensor_scalar_mul(negm[:], m_both[:], -1.0)
            nc.scalar.dma_start(
                ar_m_in.ap().rearrange("(lt p) o -> p (lt o)", p=128), m_both[:])
            nc.gpsimd.collective_compute(
                "AllReduce", ALU.max, replica_groups=RG,
                ins=[ar_m_in.ap().opt()], outs=[ar_m_out.ap().opt()])
            for lt in range(LT):
                sp0 = small.tile([128, 1], f32, name=f"sp0_{lt}")
                nc.scalar.activation(p_sb[lt][:, 0:512], score_ps[lt][0][:],
                                     AF.Exp, bias=negm[:, lt:lt + 1], accum_out=sp0[:])
                nc.scalar.activation(p_sb[lt][:, 512:1024], score_ps[lt][1][:],
                                     AF.Exp, bias=negm[:, lt:lt + 1],
                                     accum_out=s_both[:, lt:lt + 1])
                nc.vector.tensor_tensor(s_both[:, lt:lt + 1], s_both[:, lt:lt + 1],
                                        sp0[:], ALU.add)

        # ---------------- transpose p -> pT [n, l] ---------------------------
        with tc.tile_pool(name="tp", bufs=2, space="PSUM") as tpps:
            for lt in range(LT):
                for nt in range(NT):
                    tp = tpps.tile([128, 128], bf16)
                    nc.tensor.transpose(
                        tp[:], p_sb[lt][:, nt * 128:(nt + 1) * 128], ident_bf[:])
                    nc.vector.tensor_copy(pT[:, nt, lt * 128:(lt + 1) * 128], tp[:])

        # ---- phase 3 h0: MMs + plain copies BEFORE the factor chain ---------
        xb_pool2 = ctx.enter_context(tc.tile_pool(name="ph3x2", bufs=4))
        wv_pool2 = ctx.enter_context(tc.tile_pool(name="ph4w2", bufs=2))

        def wv_load2(c):
            wv_c = wv_pool2.tile([128, WVCH, DS], bf16, name="wv_c2")
            nc.sync.dma_start(wv_c[:], wvt_re[:, c * WVCH:(c + 1) * WVCH, :])
            return wv_c

        def xb_load2(c):
            xb_c = xb_pool2.tile([128, NT, XBCH * 128], bf16, name="xb_c2")
            nc.sync.dma_start(
                xb_c[:], xb_re[:, :, c * XBCH * 128:(c + 1) * XBCH * 128])
            return xb_c

        for c in (2, 3, 4, 5):
            xb_cs[c] = xb_load2(c)
        wv_cs.append(wv_load2(2))
        wv_cs.append(wv_load2(3))
        u_h = [persist.tile([128, HT, L], bf16, name=f"u_h{h}") for h in range(2)]
        ctx_c = [persist.tile([128, WVCH, L], bf16, name=f"ctx_c{c}")
                 for c in range(4)]
        s_ctx = persist.tile([128, L], bf16, name="s_ctx")
        with tc.tile_pool(name="ph3ps", bufs=4, space="PSUM") as ph3ps:
            for c in range(4):
                for i in range(XBCH):
                    dt = c * XBCH + i
                    psu = ph3ps.tile([128, L], f32)
                    for nt in range(NT):
                        nc.tensor.matmul(
                            psu[:], xb_cs[c][:, nt, i * 128:(i + 1) * 128],
                            pT[:, nt, :], start=(nt == 0), stop=(nt == NT - 1))
                    nc.vector.tensor_copy(u_h[0][:, dt, :], psu[:])

            # ---- phase 3 h1: MMs + plain copies (no factor dependency) ------
            for c in range(4, 8):
                if c >= 6:
                    xb_cs[c] = xb_load2(c)
                for i in range(XBCH):
                    dt = c * XBCH + i
                    psu = ph3ps.tile([128, L], f32)
                    for nt in range(NT):
                        nc.tensor.matmul(
                            psu[:], xb_cs[c][:, nt, i * 128:(i + 1) * 128],
                            pT[:, nt, :], start=(nt == 0), stop=(nt == NT - 1))
                    nc.vector.tensor_copy(u_h[1][:, dt - HT, :], psu[:])

            # ---- factor = exp(m_j - M); broadcast via small DRAM bounce -----
            M2 = small.tile([128, 2], f32, name="M2")
            nc.gpsimd.dma_start(
                M2[:], ar_m_out.ap().rearrange("(lt p) o -> p (lt o)", p=128))
            fac2 = small.tile([128, 2], f32, name="fac2")
            nc.vector.tensor_tensor(fac2[:], m_both[:], M2[:], ALU.subtract)
            nc.scalar.activation(fac2[:], fac2[:], AF.Exp)
            sp2 = small.tile([128, 2], f32, name="sp2")
            nc.vector.tensor_tensor(sp2[:], s_both[:], fac2[:], ALU.mult)
            nc.vector.tensor_copy(s_blk[:, 0:2], sp2[:])
            nc.scalar.dma_start(aru_re[:, DT, :], s_blk[:])

            # ---- factor broadcast via DRAM bounce (no tensor-queue ops) -----
            facb = small.tile([128, L], f32, name="facb")
            nc.scalar.dma_start(
                fac_dram.ap().rearrange("o (lt p) -> p (o lt)", p=128), fac2[:])
            fb_bc = fac_dram.ap().partition_broadcast(128)[:, 0, :]
            nc.scalar.dma_start(facb[0:64, :], fb_bc[0:64, :])
            nc.sync.dma_start(facb[64:128, :], fb_bc[64:128, :])

            # rescale both halves in place, ship as one AllReduce
            for h in range(2):
                for i in range(HT):
                    nc.vector.tensor_tensor(u_h[h][:, i, :], u_h[h][:, i, :],
                                            facb[:], ALU.mult)
                nc.scalar.dma_start(aru_re[:, h * HT:(h + 1) * HT, :], u_h[h][:])
            nc.gpsimd.collective_compute(
                "AllReduce", ALU.add, replica_groups=RG,
                ins=[ar_u_in.ap().opt()], outs=[ar_u_out.ap().opt()])
            for c in range(4):
                nc.scalar.dma_start(ctx_c[c][:],
                                    aruo_re[:, c * WVCH:(c + 1) * WVCH, :])
            nc.scalar.dma_start(s_ctx[:], aruo_re[:, DT, :])

        # keep the PE clock-gate open across the AR-u mesh (no data deps)
        with tc.tile_pool(name="warmps2", bufs=1, space="PSUM") as warmps2:
            wps2 = warmps2.tile([128, 128], f32)
            for i in range(200):
                nc.tensor.matmul(wps2[:], ident_f[:], ident_f[:],
                                 start=(i == 0), stop=(i == 199))

        # 1/S
        s_f = small.tile([128, 2], f32, name="s_f")
        nc.vector.tensor_copy(s_f[:], s_ctx[:, 0:2])
        rec2 = small.tile([128, 2], f32, name="rec2")
        nc.vector.reciprocal(rec2[:], s_f[:])

        # ---------------- phase 4: out = (ctxT/S).T @ WvT + Wv_b -------------
        with tc.tile_pool(name="ph4ps", bufs=1, space="PSUM") as ph4ps, \
             tc.tile_pool(name="ph4o", bufs=2) as out_pool:
            po = [ph4ps.tile([128, DS], f32, name=f'po{i}') for i in range(LT)]
            for c in range(4):
                for i in range(WVCH):
                    dt = c * WVCH + i
                    for lt in range(LT):
                        nc.tensor.matmul(
                            po[lt][:], ctx_c[c][:, i, lt * 128:(lt + 1) * 128],
                            wv_cs[c][:, i, :], start=(dt == 0), stop=(dt == DT - 1))
            for lt in range(LT):
                o_sb = out_pool.tile([128, DS], f32)
                nc.scalar.activation(o_sb[:], po[lt][:], AF.Copy,
                                     scale=rec2[:, lt:lt + 1])
                nc.vector.tensor_tensor(o_sb[:], o_sb[:], bias_sb[:], ALU.add)
                nc.sync.dma_start(t_out[lt * 128:(lt + 1) * 128, :], o_sb[:])

    if split_waits:
        _split_waits(nc, mybir, bass_rust)
        _NC = nc
    return nc


last_results = None


def kernel(src_prompts, query, Wk_w, Wk_b, Wv_w, Wv_b):
    global last_results
    import ml_dtypes
    from concourse.bass_utils import run_bass_kernel_spmd

    nc = _build()

    x = np.ascontiguousarray(np.asarray(src_prompts, dtype=np.float32)[0])
    q = np.asarray(query, dtype=np.float32)
    wk = np.asarray(Wk_w, dtype=np.float32)
    wv = np.asarray(Wv_w, dtype=np.float32)
    wvb = np.asarray(Wv_b, dtype=np.float32)
    # Wk_b shifts every score row by a constant -> cancels in softmax.

    qT = np.ascontiguousarray(q.T)
    in_maps = []
    for j in range(NCORES):
        ns, ds = slice(j * NS, (j + 1) * NS), slice(j * DS, (j + 1) * DS)
        xj = x[ns]
        in_maps.append({
            "qTs": qT.astype(np.float16),
            "wk": np.ascontiguousarray(wk[:, ds].astype(np.float16)),
            "xT": np.ascontiguousarray(xj.T.astype(np.float16)),
            "xb": np.ascontiguousarray(xj.astype(ml_dtypes.bfloat16)),
            "wvT": np.ascontiguousarray(wv[ds].T.astype(ml_dtypes.bfloat16)),
            "wvb": np.ascontiguousarray(wvb[ds][None, :].astype(ml_dtypes.bfloat16)),
        })

    res = run_bass_kernel_spmd(nc, in_maps, core_ids=list(range(NCORES)))
    last_results = res
    out = np.concatenate([res.results[j]["out"] for j in range(NCORES)], axis=1)
    return out[None, :, :]


# revision 10
# speedup vs baseline: 1.0587x; 1.0587x over previous
"""Trainium2 Bass kernel for nn_AttentionProjector (8-core SPMD), v3.

Math: out = softmax(q @ (x@Wk.T).T) @ (x@Wv.T + Wv_b)
Rewritten (FLOP reduction):
    scores = (q @ Wk) @ x.T      (Wk_b cancels in softmax)
    out    = (softmax(scores) @ x) @ Wv.T + Wv_b

Structure (8 cores):
  warm-up : tiny AllReduce at t=0 absorbs the ~100us ncfw first-collective
            cold start while DMAs and phase 1 run.
  phase 1 : q'T slice via Wk[:,ds_j] columns -> AllGather q'T (f32r
            end-to-end, HWDGE readback).
  phase 2 : scores[l, n_j] = q'T.T @ xT_j, f32r, N sharded.
  softmax : LOCAL row max -> p = exp(s - m_j) immediately (bf16); tiny
            AllGather of m_j overlaps phase 3; online-softmax rescale by
            exp(m_j - M) applied to u/s before the AllReduce-add.
  phase 3 : uT_j = x_j.T @ p_j.T in bf16. h0 matmuls+copies are emitted
            BEFORE the factor chain so the tensor/vector FIFOs never wait
            on the m AllGather; factor broadcast via a small DRAM bounce
            (no PE involvement).
  AR-u    : ONE AllReduce-add, bf16, u halves + s' in a single payload.
  phase 4 : out[:, do_j] = (ctxT/S).T @ Wv[do_j,:].T + Wv_b, all bf16.

Precision: score path (qTs, wk, xT, AG-q') stays f32r -- bf16 there flips
softmax argmaxes past tolerance. Values path (x, p, u, Wv) is bf16
(host sim rel 5.5e-3; measured on HW 2.9e-3 in v2).
"""

import numpy as np

L = 256          # query rows
D = 4096         # d_in == d_out
N = 8192         # tokens
NCORES = 8
NS = N // NCORES     # 1024 tokens per core
DS = D // NCORES     # 512 dout per core

LT = L // 128        # 2 l-tiles
DT = D // 128        # 32 d-tiles
NT = NS // 128       # 8 local n-tiles
HT = DT // 2         # 16 d-tiles per u half

_MAX_WAITS = 1


def _split_waits(nc, mybir, bass_rust):
    """Walrus in this container allows only one sync-wait per instruction;
    move excess waits onto preceding same-engine no-ops."""
    for bb in nc.main_func.blocks:
        new_list = []
        for ins in bb.instructions:
            si = ins.sync_info
            waits = list(si.on_wait) if si is not None else []
            if len(waits) > _MAX_WAITS:
                for i in range(_MAX_WAITS, len(waits), _MAX_WAITS):
                    nop = mybir.InstNoOp(name=f"{ins.name}-wsplit{i}", ins=[], outs=[])
                    nop.engine = ins.engine
                    nop.sync_info = bass_rust.SyncInfo(
                        on_wait=waits[i:i + _MAX_WAITS], on_update=[])
                    new_list.append(nop)
                ins.sync_info = bass_rust.SyncInfo(
                    on_wait=waits[:_MAX_WAITS], on_update=si.on_update)
            new_list.append(ins)
        bb.instructions[:] = new_list


_NC = None


def _build(split_waits=True):
    global _NC
    if _NC is not None and split_waits:
        return _NC
    import bass_rust
    import concourse.bass as bass
    import concourse.mybir as mybir
    import concourse.tile as tile
    from concourse.masks import make_identity
    from contextlib import ExitStack

    f32 = mybir.dt.float32
    f32r = mybir.dt.float32r
    bf16 = mybir.dt.bfloat16
    AF = mybir.ActivationFunctionType
    AX = mybir.AxisListType
    ALU = mybir.AluOpType
    RG = [list(range(NCORES))]

    f16 = mybir.dt.float16
    nc = bass.Bass()

    # per-core external I/O
    t_qts = nc.dram_tensor("qTs", [D, L], f16, kind="ExternalInput")
    t_wk = nc.dram_tensor("wk", [D, DS], f16, kind="ExternalInput")
    t_xt = nc.dram_tensor("xT", [D, NS], f16, kind="ExternalInput")
    t_xb = nc.dram_tensor("xb", [NS, D], bf16, kind="ExternalInput")
    t_wvt = nc.dram_tensor("wvT", [D, DS], bf16, kind="ExternalInput")
    t_wvb = nc.dram_tensor("wvb", [1, DS], bf16, kind="ExternalInput")
    t_out = nc.dram_tensor("out", [L, DS], f32, kind="ExternalOutput")

    # collective bounce buffers (input Local, output Shared)
    warm_in = nc.dram_tensor("warm_in", [1, 128], f32)
    warm_out = nc.dram_tensor("warm_out", [NCORES, 128], f32, addr_space="Shared")
    ar_q_in = nc.dram_tensor("ar_q_in", [DS, L], f16)
    ar_q_out = nc.dram_tensor("ar_q_out", [D, L], f16, addr_space="Shared")
    ar_m_in = nc.dram_tensor("ar_m_in", [L, 1], f32)
    ar_m_out = nc.dram_tensor("ar_m_out", [NCORES * L, 1], f32, addr_space="Shared")
    fac_dram = nc.dram_tensor("fac_dram", [1, L], f32)
    # u: 32 d-tile blocks + one s' block, single AllReduce payload
    ar_u_in = nc.dram_tensor("ar_u_in", [(DT + 1) * 128, L], bf16)
    ar_u_out = nc.dram_tensor("ar_u_out", [(DT + 1) * 128, L], bf16,
                              addr_space="Shared")

    qts_re = t_qts.ap().rearrange("(kt p) l -> p kt l", p=128)   # [128, 32, 256]
    wk_re = t_wk.ap().rearrange("(kt p) d -> p kt d", p=128)     # [128, 32, 512]
    xt_re = t_xt.ap().rearrange("(dt p) n -> p dt n", p=128)     # [128, 32, 1024]
    xb_re = t_xb.ap().rearrange("(nt p) d -> p nt d", p=128)     # [128, 8, 4096]
    wvt_re = t_wvt.ap().rearrange("(dt p) o -> p dt o", p=128)   # [128, 32, 512]
    arq_re = ar_q_in.ap().rearrange("(dt p) l -> p dt l", p=128)
    arqo_re = ar_q_out.ap().rearrange("(dt p) l -> p dt l", p=128)
    aru_re = ar_u_in.ap().rearrange("(t p) l -> p t l", p=128)
    aruo_re = ar_u_out.ap().rearrange("(t p) l -> p t l", p=128)

    with ExitStack() as ctx:
        tc = ctx.enter_context(tile.TileContext(nc))
        const = ctx.enter_context(tc.tile_pool(name="const", bufs=1))
        small = ctx.enter_context(tc.tile_pool(name="small", bufs=1))
        persist = ctx.enter_context(tc.tile_pool(name="persist", bufs=1))

        # ---- warm-up: tiny collective absorbs ncfw cold-start ----------------
        nc.gpsimd.collective_compute(
            "AllGather", ALU.bypass, replica_groups=RG,
            ins=[warm_in.ap().opt()], outs=[warm_out.ap().opt()])

        ident_bf = const.tile([128, 128], bf16)
        make_identity(nc, ident_bf[:])
        ident_f = const.tile([128, 128], f32)
        make_identity(nc, ident_f[:])
        ones1 = const.tile([1, 128], f32)
        nc.vector.memset(ones1[:], 1.0)
        bias_sb = const.tile([128, DS], bf16)
        wvb_sb = const.tile([1, DS], bf16)
        nc.scalar.dma_start(wvb_sb[:], t_wvb.ap())
        ones1b = const.tile([1, 128], bf16)
        nc.vector.memset(ones1b[:], 1.0)
        s_blk = const.tile([128, L], bf16)       # s' payload block (zeros + 2 cols)
        nc.vector.memset(s_blk[:], 0.0)

        # PE clock-gate warm-up during the first input DMAs
        with tc.tile_pool(name="warmps", bufs=1, space="PSUM") as warmps:
            wps = warmps.tile([128, 128], f32)
            for i in range(28):
                nc.tensor.matmul(wps[:], ident_bf[:], ident_bf[:],
                                 start=(i == 0), stop=(i == 27))
            # bias broadcast [1,DS] -> [128,DS] via rank-1 matmul
            bps = warmps.tile([128, DS], f32)
            nc.tensor.matmul(bps[:], ones1b[:], wvb_sb[:], start=True, stop=True)
            nc.vector.tensor_copy(bias_sb[:], bps[:])

        # persistent across phases
        pT = persist.tile([128, NT, L], bf16)        # p.T (0.5MB)

        # streamed values path on the scalar queue (pools opened before ph2xt
        # so they land outside the xT stream's space)
        xb_pool = ctx.enter_context(tc.tile_pool(name="ph3x", bufs=2))
        wv_pool = ctx.enter_context(tc.tile_pool(name="ph4w", bufs=2))
        XBCH = 4                     # d-tiles per xb chunk (1MB)
        WVCH = 8                     # d-tiles per wv chunk (1MB)

        def xb_load(c, eng=None):
            xb_c = xb_pool.tile([128, NT, XBCH * 128], bf16, name="xb_c")
            (eng or nc.sync).dma_start(
                xb_c[:], xb_re[:, :, c * XBCH * 128:(c + 1) * XBCH * 128])
            return xb_c

        def wv_load(c, eng=None):
            wv_c = wv_pool.tile([128, WVCH, DS], bf16, name="wv_c")
            (eng or nc.sync).dma_start(wv_c[:], wvt_re[:, c * WVCH:(c + 1) * WVCH, :])
            return wv_c

        # ---------------- phase 1: q'T partial = Wk[:,ds_j].T @ q.T ----------
        with tc.tile_pool(name="ph1q", bufs=2) as ph1q, \
             tc.tile_pool(name="ph1wk", bufs=4) as ph1wk, \
             tc.tile_pool(name="ph1ps", bufs=1, space="PSUM") as ph1ps:
            def qts_load(qc):
                qts_c = ph1q.tile([128, 8, L], f16, name="qts_c")
                nc.sync.dma_start(qts_c[:], qts_re[:, qc * 8:(qc + 1) * 8, :])
                return qts_c
            qts_cs = [qts_load(0)]
            qpT_loc = ph1q.tile([128, 4, L], f16, name="qpT_loc")
            ps4 = [ph1ps.tile([128, L], f32, name=f"ph1ps{i}") for i in range(4)]
            KCH = 4                                  # k-tiles per wk chunk (1MB)
            for kc in range(DT // KCH):
                wk_c = ph1wk.tile([128, KCH, DS], f16, name="wk_c")
                nc.sync.dma_start(wk_c[:], wk_re[:, kc * KCH:(kc + 1) * KCH, :])
                if kc % 2 == 0 and kc // 2 + 1 < 4:
                    qts_cs.append(qts_load(kc // 2 + 1))
                for i in range(KCH):
                    kt = kc * KCH + i
                    for dtl in range(4):
                        nc.tensor.matmul(
                            ps4[dtl][:], wk_c[:, i, dtl * 128:(dtl + 1) * 128],
                            qts_cs[kt // 8][:, kt % 8, :],
                            start=(kt == 0), stop=(kt == DT - 1))
            for dtl in range(4):
                nc.vector.tensor_copy(qpT_loc[:, dtl, :], ps4[dtl][:])
            nc.scalar.dma_start(arq_re, qpT_loc[:])
            nc.gpsimd.collective_compute(
                "AllGather", ALU.bypass, replica_groups=RG,
                ins=[ar_q_in.ap().opt()], outs=[ar_q_out.ap().opt()])

        # early value-path loads (fill the collective cold-start window)
        xb_cs = {0: xb_load(0, nc.scalar), 1: xb_load(1, nc.scalar)}
        wv_cs = [wv_load(0, nc.scalar), wv_load(1, nc.scalar)]
        # q'T readback: f32 -> f32r cast (SWDGE) on gpsimd, 4 chunks
        qpT = persist.tile([128, DT, L], f16, name="qpT")
        for rc in range(4):
            nc.scalar.dma_start(qpT[:, rc * 8:(rc + 1) * 8, :],
                                arqo_re[:, rc * 8:(rc + 1) * 8, :])

        # ---------------- phase 2: scores[l, n_j] ----------------------------
        XCH = 8                      # d-tiles per xT chunk (4MB)
        m_both = small.tile([128, 2], f32, name="m_both")
        s_both = small.tile([128, 2], f32, name="s_both")
        p_sb = [persist.tile([128, NS], bf16, name=f'p_sb{i}') for i in range(LT)]
        with tc.tile_pool(name="ph2xt", bufs=4) as xt_pool, \
             tc.tile_pool(name="ph2sc", bufs=1, space="PSUM") as scps_pool:
            score_ps = [[scps_pool.tile([128, 512], f32, name=f'score{i}_{k}')
                         for k in range(2)] for i in range(LT)]
            for c in range(DT // XCH):
                xt_c = xt_pool.tile([128, XCH, NS], f16, name="xt_c")
                nc.sync.dma_start(xt_c[:], xt_re[:, c * XCH:(c + 1) * XCH, :])
                for i in range(XCH):
                    dt = c * XCH + i
                    for lt in range(LT):
                        for nch in range(2):
                            nc.tensor.matmul(
                                score_ps[lt][nch][:],
                                qpT[:, dt, lt * 128:(lt + 1) * 128],
                                xt_c[:, i, nch * 512:(nch + 1) * 512],
                                start=(dt == 0), stop=(dt == DT - 1))

            # local row max, ship m_j, p = exp(s - m_j) immediately
            negm = small.tile([128, 2], f32, name="negm")
            for lt in range(LT):
                mtmp = small.tile([128, 1], f32, name=f"mtmp{lt}")
                nc.vector.tensor_reduce(mtmp[:], score_ps[lt][0][:], axis=AX.X, op=ALU.max)
                nc.vector.tensor_reduce(m_both[:, lt:lt + 1], score_ps[lt][1][:],
                                        axis=AX.X, op=ALU.max)
                nc.vector.tensor_tensor(m_both[:, lt:lt + 1], m_both[:, lt:lt + 1],
                                        mtmp[:], ALU.max)
            nc.vector.tensor_scalar_mul(negm[:], m_both[:], -1.0)
            nc.scalar.dma_start(
                ar_m_in.ap().rearrange("(lt p) o -> p (lt o)", p=128), m_both[:])
            nc.gpsimd.collective_compute(
                "AllGather", ALU.bypass, replica_groups=RG,
                ins=[ar_m_in.ap().opt()], outs=[ar_m_out.ap().opt()])
            for lt in range(LT):
                sp0 = small.tile([128, 1], f32, name=f"sp0_{lt}")
                nc.scalar.activation(p_sb[lt][:, 0:512], score_ps[lt][0][:],
                                     AF.Exp, bias=negm[:, lt:lt + 1], accum_out=sp0[:])
                nc.scalar.activation(p_sb[lt][:, 512:1024], score_ps[lt][1][:],
                                     AF.Exp, bias=negm[:, lt:lt + 1],
                                     accum_out=s_both[:, lt:lt + 1])
                nc.vector.tensor_tensor(s_both[:, lt:lt + 1], s_both[:, lt:lt + 1],
                                        sp0[:], ALU.add)

        # ---------------- transpose p -> pT [n, l] ---------------------------
        with tc.tile_pool(name="tp", bufs=2, space="PSUM") as tpps:
            for lt in range(LT):
                for nt in range(NT):
                    tp = tpps.tile([128, 128], bf16)
                    nc.tensor.transpose(
                        tp[:], p_sb[lt][:, nt * 128:(nt + 1) * 128], ident_bf[:])
                    nc.vector.tensor_copy(pT[:, nt, lt * 128:(lt + 1) * 128], tp[:])

        # ---- phase 3 h0: MMs + plain copies BEFORE the factor chain ---------
        xb_pool2 = ctx.enter_context(tc.tile_pool(name="ph3x2", bufs=4))
        wv_pool2 = ctx.enter_context(tc.tile_pool(name="ph4w2", bufs=2))

        def wv_load2(c):
            wv_c = wv_pool2.tile([128, WVCH, DS], bf16, name="wv_c2")
            nc.sync.dma_start(wv_c[:], wvt_re[:, c * WVCH:(c + 1) * WVCH, :])
            return wv_c

        def xb_load2(c):
            xb_c = xb_pool2.tile([128, NT, XBCH * 128], bf16, name="xb_c2")
            nc.sync.dma_start(
                xb_c[:], xb_re[:, :, c * XBCH * 128:(c + 1) * XBCH * 128])
            return xb_c

        for c in (2, 3, 4, 5):
            xb_cs[c] = xb_load2(c)
        wv_cs.append(wv_load2(2))
        wv_cs.append(wv_load2(3))
        u_h = [persist.tile([128, HT, L], bf16, name=f"u_h{h}") for h in range(2)]
        ctx_c = [persist.tile([128, WVCH, L], bf16, name=f"ctx_c{c}")
                 for c in range(4)]
        s_ctx = persist.tile([128, L], bf16, name="s_ctx")
        with tc.tile_pool(name="ph3ps", bufs=4, space="PSUM") as ph3ps:
            for c in range(4):
                for i in range(XBCH):
                    dt = c * XBCH + i
                    psu = ph3ps.tile([128, L], f32)
                    for nt in range(NT):
                        nc.tensor.matmul(
                            psu[:], xb_cs[c][:, nt, i * 128:(i + 1) * 128],
                            pT[:, nt, :], start=(nt == 0), stop=(nt == NT - 1))
                    nc.vector.tensor_copy(u_h[0][:, dt, :], psu[:])

            # ---- phase 3 h1: MMs + plain copies (no factor dependency) ------
            for c in range(4, 8):
                if c >= 6:
                    xb_cs[c] = xb_load2(c)
                for i in range(XBCH):
                    dt = c * XBCH + i
                    psu = ph3ps.tile([128, L], f32)
                    for nt in range(NT):
                        nc.tensor.matmul(
                            psu[:], xb_cs[c][:, nt, i * 128:(i + 1) * 128],
                            pT[:, nt, :], start=(nt == 0), stop=(nt == NT - 1))
                    nc.vector.tensor_copy(u_h[1][:, dt - HT, :], psu[:])

            # ---- factor = exp(m_j - M); broadcast via small DRAM bounce -----
            ms_sb = small.tile([128, 2 * NCORES], f32, name="ms_sb")
            nc.gpsimd.dma_start(
                ms_sb[:], ar_m_out.ap().rearrange("(r lt p) o -> p (r lt o)",
                                                  p=128, lt=LT))
            M2 = small.tile([128, 2], f32, name="M2")
            for lt in range(LT):
                nc.vector.tensor_copy(M2[:, lt:lt + 1], ms_sb[:, lt:lt + 1])
                for r in range(1, NCORES):
                    nc.vector.tensor_tensor(M2[:, lt:lt + 1], M2[:, lt:lt + 1],
                                            ms_sb[:, 2 * r + lt:2 * r + lt + 1],
                                            ALU.max)
            fac2 = small.tile([128, 2], f32, name="fac2")
            nc.vector.tensor_tensor(fac2[:], m_both[:], M2[:], ALU.subtract)
            nc.scalar.activation(fac2[:], fac2[:], AF.Exp)
            sp2 = small.tile([128, 2], f32, name="sp2")
            nc.vector.tensor_tensor(sp2[:], s_both[:], fac2[:], ALU.mult)
            nc.vector.tensor_copy(s_blk[:, 0:2], sp2[:])
            nc.scalar.dma_start(aru_re[:, DT, :], s_blk[:])

            # ---- factor broadcast via PE (tensor queue after h1 MMs) --------
            facb = small.tile([128, L], f32, name="facb")
            with tc.tile_pool(name="facps", bufs=1, space="PSUM") as facps_pool:
                fb_ps = facps_pool.tile([128, L], f32, name="fb_ps")
                for lt in range(LT):
                    fac_ps = facps_pool.tile([1, 128], f32, name=f"fac_ps{lt}")
                    nc.tensor.transpose(fac_ps[:], fac2[:, lt:lt + 1], ident_f[:])
                    facr = small.tile([1, 128], f32, name=f"facr{lt}")
                    nc.vector.tensor_copy(facr[:], fac_ps[:])
                    nc.tensor.matmul(fb_ps[:, lt * 128:(lt + 1) * 128],
                                     ones1[:], facr[:], start=True, stop=True)
                nc.vector.tensor_copy(facb[:], fb_ps[:])

            # rescale both halves in place, ship as one AllReduce
            for h in range(2):
                for i in range(HT):
                    nc.vector.tensor_tensor(u_h[h][:, i, :], u_h[h][:, i, :],
                                            facb[:], ALU.mult)
                nc.scalar.dma_start(aru_re[:, h * HT:(h + 1) * HT, :], u_h[h][:])
            nc.gpsimd.collective_compute(
                "AllReduce", ALU.add, replica_groups=RG,
                ins=[ar_u_in.ap().opt()], outs=[ar_u_out.ap().opt()])
            for c in range(4):
                nc.scalar.dma_start(ctx_c[c][:],
                                    aruo_re[:, c * WVCH:(c + 1) * WVCH, :])
            nc.scalar.dma_start(s_ctx[:], aruo_re[:, DT, :])

        # keep the PE clock-gate open across the AR-u mesh (no data deps)
        with tc.tile_pool(name="warmps2", bufs=1, space="PSUM") as warmps2:
            wps2 = warmps2.tile([128, 128], f32)
            for i in range(200):
                nc.tensor.matmul(wps2[:], ident_f[:], ident_f[:],
                                 start=(i == 0), stop=(i == 199))

        # 1/S
        s_f = small.tile([128, 2], f32, name="s_f")
        nc.vector.tensor_copy(s_f[:], s_ctx[:, 0:2])
        rec2 = small.tile([128, 2], f32, name="rec2")
        nc.vector.reciprocal(rec2[:], s_f[:])

        # ---------------- phase 4: out = (ctxT/S).T @ WvT + Wv_b -------------
        with tc.tile_pool(name="ph4ps", bufs=1, space="PSUM") as ph4ps, \
             tc.tile_pool(name="ph4o", bufs=2) as out_pool:
            po = [ph4ps.tile([128, DS], f32, name=f'po{i}') for i in range(LT)]
            for c in range(4):
                for i in range(WVCH):
                    dt = c * WVCH + i
                    for lt in range(LT):
                        nc.tensor.matmul(
                            po[lt][:], ctx_c[c][:, i, lt * 128:(lt + 1) * 128],
                            wv_cs[c][:, i, :], start=(dt == 0), stop=(dt == DT - 1))
            for lt in range(LT):
                o_sb = out_pool.tile([128, DS], f32)
                nc.scalar.activation(o_sb[:], po[lt][:], AF.Copy,
                                     scale=rec2[:, lt:lt + 1])
                nc.vector.tensor_tensor(o_sb[:], o_sb[:], bias_sb[:], ALU.add)
                nc.sync.dma_start(t_out[lt * 128:(lt + 1) * 128, :], o_sb[:])

    if split_waits:
        _split_waits(nc, mybir, bass_rust)
        _NC = nc
    return nc


last_results = None


def kernel(src_prompts, query, Wk_w, Wk_b, Wv_w, Wv_b):
    global last_results
    import ml_dtypes
    from concourse.bass_utils import run_bass_kernel_spmd

    nc = _build()

    x = np.ascontiguousarray(np.asarray(src_prompts, dtype=np.float32)[0])
    q = np.asarray(query, dtype=np.float32)
    wk = np.asarray(Wk_w, dtype=np.float32)
    wv = np.asarray(Wv_w, dtype=np.float32)
    wvb = np.asarray(Wv_b, dtype=np.float32)
    # Wk_b shifts every score row by a constant -> cancels in softmax.

    qT = np.ascontiguousarray(q.T)
    in_maps = []
    for j in range(NCORES):
        ns, ds = slice(j * NS, (j + 1) * NS), slice(j * DS, (j + 1) * DS)
        xj = x[ns]
        in_maps.append({
            "qTs": qT.astype(np.float16),
            "wk": np.ascontiguousarray(wk[:, ds].astype(np.float16)),
            "xT": np.ascontiguousarray(xj.T.astype(np.float16)),
            "xb": np.ascontiguousarray(xj.astype(ml_dtypes.bfloat16)),
            "wvT": np.ascontiguousarray(wv[ds].T.astype(ml_dtypes.bfloat16)),
            "wvb": np.ascontiguousarray(wvb[ds][None, :].astype(ml_dtypes.bfloat16)),
        })

    res = run_bass_kernel_spmd(nc, in_maps, core_ids=list(range(NCORES)))
    last_results = res
    out = np.concatenate([res.results[j]["out"] for j in range(NCORES)], axis=1)
    return out[None, :, :]
